# revision 28
# baseline (speedup 1.0000x reference)
"""Trainium2 Bass kernel for nn_LowRankDynamicConv.

Math (per sample b):
  combined = [phrase_slot[b] | eos]                       [N, 2C]
  h        = relu(combined @ W1 + b1)                     [N, 4C]
  proj     = (h @ W2 + b2) viewed as [N*C, R]             [4096, 32]
  y        = x[b] @ proj   with x[b] = context_emb[b] as  [T, N*C]
  out[t]   = relu(LN(sum_{k,j} y[t + j - pad_k] @ M_kj + bo))
  where M_kj[r, co] = sum_d kjoin[kj, r, d] * Wo[k_block*C + d, co]
The 9 conv taps are stacked 4-per-128-partitions (shifted copies of y), so
emit is 3 matmuls per 128-row output tile; bo rides along as a rank-1
ones-row contraction in the third matmul.

Sharding: data-parallel over batch, 2 samples per core, weights replicated
(cross-core collectives measure ~70us latency on this runtime, so W2 is
streamed in full per core instead of sharded). All heavy streams travel as
bfloat16 with >=2KB DMA descriptors; x is pre-transposed on the host to
[NCF, T] so the contraction dim lands on SBUF partitions with no on-chip
transposes, streamed in nc-group chunks that the stage-3 accumulation
chases. Stores and reshard ride the scalar/gpsimd DMA queues so they never
queue behind the big streams on the sync queue.
"""
import sys

sys.path.insert(0, "/opt/trn_rl_repo")

import ml_dtypes
import numpy as np

import concourse.bass as bass  # noqa: F401  (bass types used via bacc)
import concourse.mybir as mybir
import concourse.tile as tile
from concourse import bacc
from concourse.bass_utils import run_bass_kernel_spmd
from concourse.masks import make_identity

F32 = mybir.dt.float32
BF16 = mybir.dt.bfloat16
RELU = mybir.ActivationFunctionType.Relu
SQRT = mybir.ActivationFunctionType.Sqrt
IDENT = mybir.ActivationFunctionType.Identity
BF = ml_dtypes.bfloat16

NCORES = 8
BPC = 2                    # samples per core
T, N, C, R = 1024, 16, 256, 32
NCF = N * C                # 4096 flattened (n, c) contraction dim
CH = NCF // 128            # 32 nc-chunks of 128
NG = 8                     # x stream groups per sample
GCH = CH // NG             # nc-chunks per group (8)
HALF = 512                 # stage-3 PSUM free-dim chunk (one PSUM bank)
PAD = 2                    # max conv pad (k=5)
YW = T + 2 * PAD           # padded y width, 1028
NJ = 9                     # total (kernel, tap) count
YSPL = 504                 # yk build split point (part 1 safe with half 0 only)
# (wo-block, temporal offset) per fused tap, in k1 | k3 | k5 order
JOFF = [(0, 0), (1, -1), (1, 0), (1, 1), (2, -2), (2, -1), (2, 0), (2, 1), (2, 2)]


def _broadcast_ap(ap, parts):
    """DMA access pattern replicating a 1D/2D DRAM tensor across `parts` partitions."""
    a = ap
    return bass.AP(tensor=a.tensor, offset=a.offset, ap=[[0, parts]] + list(a.ap))


def _build():
    nc = bacc.Bacc("TRN2", num_devices=NCORES)

    xbT = nc.dram_tensor("xbT", [BPC, NCF, T], BF16, kind="ExternalInput")
    phrase = nc.dram_tensor("phrase", [BPC * N, C], BF16, kind="ExternalInput")
    eos = nc.dram_tensor("eos", [C], BF16, kind="ExternalInput")
    w1 = nc.dram_tensor("w1", [2 * C, 4 * C], BF16, kind="ExternalInput")
    b1 = nc.dram_tensor("b1", [4 * C], F32, kind="ExternalInput")
    w2 = nc.dram_tensor("w2", [4 * C, C * R], BF16, kind="ExternalInput")
    b2 = nc.dram_tensor("b2", [C * R], F32, kind="ExternalInput")
    kjoin = nc.dram_tensor("kjoin", [NJ, R, C], BF16, kind="ExternalInput")
    wo = nc.dram_tensor("wo", [3 * C, C], BF16, kind="ExternalInput")
    lnp = nc.dram_tensor("lnp", [3, C], F32, kind="ExternalInput")
    out = nc.dram_tensor("out", [BPC, 2, 128, 4 * C], BF16,
                         kind="ExternalOutput")

    with tile.TileContext(nc) as tc:
        with tc.tile_pool(name="keep", bufs=1) as keep, \
             tc.tile_pool(name="pXg", bufs=10) as pXg, \
             tc.tile_pool(name="dram", bufs=1, space="DRAM") as dram:
            ident = keep.tile([128, 128], BF16)
            make_identity(nc, ident)

            # LN params + output bias (gamma|beta|bo), one broadcast DMA
            lnsb = keep.tile([128, 3, C], F32)
            nc.sync.dma_start(lnsb, _broadcast_ap(lnp[:, :], 128))
            gsb = lnsb[:, 0, :]
            bsb = lnsb[:, 1, :]
            bosb = lnsb[:, 2, :]
            epsb = keep.tile([128, 1], F32)
            nc.vector.memset(epsb, 1e-5)
            # b2 regrouped to the post-reshard proj layout: [c%128, c-half, r]
            b2v = keep.tile([128, 2, R], F32)
            nc.sync.dma_start(b2v, b2[:].rearrange("(c2 p r) -> p c2 r", p=128, r=R))
            # bf16 gamma/beta for the 2x-rate post-normalize ops
            gb16 = keep.tile([128, C], BF16)
            nc.vector.tensor_copy(gb16, gsb)
            bb16 = keep.tile([128, C], BF16)
            nc.vector.tensor_copy(bb16, bsb)

            # y^T buffers, one per sample: [r=32 part, zero-padded t] bf16
            ysb = []
            for b in range(BPC):
                y = keep.tile([R, YW], BF16, name=f"ysb{b}")
                nc.vector.memset(y[:, 0:PAD], 0.0)
                nc.vector.memset(y[:, YW - PAD:YW], 0.0)
                ysb.append(y)

            # stage-3 lhsT tiles [c%128 part, (b, c-half, n), r] bf16
            projf = keep.tile([128, BPC * CH, R], BF16)
            # fused conv+output weights M_kj [r part, tap, co], plus stacked
            # rhs tiles for the 3-matmul emit: taps 0-3 | taps 4-7 | tap 8+bo
            msb = keep.tile([R, NJ, C], BF16)
            mst0 = keep.tile([128, C], BF16)
            mst1 = keep.tile([128, C], BF16)
            mcat = keep.tile([R + 1, C], BF16)

            # ---- phase A -------------------------------------------------------
            with tc.tile_pool(name="pA", bufs=1) as pA, \
                 tc.tile_pool(name="pW2", bufs=5) as pW2, \
                 tc.tile_pool(name="pAs", bufs=2) as pAs, \
                 tc.tile_pool(name="psA", bufs=2, space="PSUM") as psA, \
                 tc.tile_pool(name="psW", bufs=2, space="PSUM") as psW:
                # combined^T [c2%128 part, ko, bn] bf16
                phsb = pA.tile([BPC * N, C], BF16)
                nc.sync.dma_start(phsb, phrase[:, :])
                eossb = pA.tile([128, 2], BF16)
                nc.sync.dma_start(eossb, eos[:].rearrange("(o p) -> p o", p=128))
                combT = pA.tile([128, 4, BPC * N], BF16)
                for ko in range(2):
                    pt = psA.tile([128, BPC * N], BF16, tag="t")
                    nc.tensor.transpose(pt, phsb[:, ko * 128:(ko + 1) * 128],
                                        ident[:BPC * N, :BPC * N])
                    nc.vector.tensor_copy(combT[:, ko, :], pt)
                for o in range(2):
                    nc.vector.tensor_copy(
                        combT[:, 2 + o, :],
                        eossb[:, o:o + 1].to_broadcast((128, BPC * N)))

                # W1 [c2%128, ko, m]; b1 -> [m%128, mo]
                w1sb = pA.tile([128, 4, 4 * C], BF16)
                nc.sync.dma_start(w1sb, w1[:, :].rearrange("(ko p) m -> p ko m", p=128))
                b1sb = pA.tile([128, 8], F32)
                nc.sync.dma_start(b1sb, b1[:].rearrange("(mo p) -> p mo", p=128))

                # h^T [m%128 part, mo, bn] = relu(W1^T combined + b1), bf16
                hT = pA.tile([128, 8, BPC * N], BF16)
                for mo in range(8):
                    ph = psA.tile([128, BPC * N], F32, tag="t")
                    for ko in range(4):
                        nc.tensor.matmul(ph, w1sb[:, ko, mo * 128:(mo + 1) * 128],
                                         combT[:, ko, :],
                                         start=(ko == 0), stop=(ko == 3))
                    nc.scalar.activation(out=hT[:, mo, :], in_=ph, func=RELU,
                                         bias=b1sb[:, mo:mo + 1], scale=1.0)

                # M_kj = kjoin_kj @ Wo_block: transpose kjoin taps, then contract
                kjf = pA.tile([R, NJ, C], BF16)
                nc.sync.dma_start(kjf, kjoin[:, :, :].rearrange("j r d -> r j d"))
                wof = pA.tile([128, 6, C], BF16)
                nc.sync.dma_start(wof, wo[:, :].rearrange("(fc p) co -> p fc co", p=128))
                kjT = pA.tile([128, 2 * NJ, R], BF16)
                for ji in range(NJ):
                    for dc in range(2):
                        pt = psA.tile([128, R], BF16, tag="t")
                        nc.tensor.transpose(pt, kjf[:, ji, dc * 128:(dc + 1) * 128],
                                            ident[:R, :R])
                        nc.vector.tensor_copy(kjT[:, ji * 2 + dc, :], pt)
                for ji, (kb, _off) in enumerate(JOFF):
                    pm = psA.tile([R, C], F32, tag="t")
                    for dc in range(2):
                        nc.tensor.matmul(pm, kjT[:, ji * 2 + dc, :],
                                         wof[:, kb * 2 + dc, :],
                                         start=(dc == 0), stop=(dc == 1))
                    nc.vector.tensor_copy(msb[:, ji, :], pm)
                # stack the emit rhs: taps 0-3 / 4-7 on 128 partitions; tap 8
                # plus the bo row on 33 (gpsimd queue: tiny, off the big FIFO)
                for q in range(4):
                    nc.gpsimd.dma_start(mst0[q * R:(q + 1) * R, :], msb[:, q, :])
                    nc.gpsimd.dma_start(mst1[q * R:(q + 1) * R, :], msb[:, 4 + q, :])
                nc.gpsimd.dma_start(mcat[0:R, :], msb[:, 8, :])
                nc.vector.tensor_copy(mcat[R:R + 1, :], bosb[0:1, :])

                # proj rows [bn, (c r)] -> DRAM scratch (bf16), 1024-col blocks
                # (2KB DMA lines keep the W2 stream at full rate)
                scratch = dram.tile([BPC * N, C * R], BF16)
                for j8 in range(8):
                    w2sb = pW2.tile([128, 8, 1024], BF16, tag="w2")
                    nc.sync.dma_start(
                        w2sb, w2[:, j8 * 1024:(j8 + 1) * 1024]
                        .rearrange("(ko p) q -> p ko q", p=128))
                    pjsb = pAs.tile([BPC * N, 1024], BF16, tag="pjsb")
                    for q2 in range(2):
                        pp = psW.tile([BPC * N, 512], F32, tag="pj")
                        for ko in range(8):
                            nc.tensor.matmul(pp, hT[:, ko, :],
                                             w2sb[:, ko, q2 * 512:(q2 + 1) * 512],
                                             start=(ko == 0), stop=(ko == 7))
                        nc.vector.tensor_copy(pjsb[:, q2 * 512:(q2 + 1) * 512], pp)
                    nc.scalar.dma_start(scratch[:, j8 * 1024:(j8 + 1) * 1024],
                                        pjsb)

            # reshard proj on the gpsimd DMA queue: its wait on the scratch
            # writes must not stall the sync queue where x is about to issue
            for b in range(BPC):
                for c2 in range(2):
                    dst = projf[:, b * CH + c2 * N:b * CH + c2 * N + N, :]
                    nc.gpsimd.dma_start(
                        dst,
                        scratch[b * N:(b + 1) * N, c2 * 4096:(c2 + 1) * 4096]
                        .rearrange("n (p r) -> p n r", p=128, r=R))
                    nc.vector.tensor_add(
                        dst, dst, b2v[:, c2:c2 + 1, :].to_broadcast((128, N, R)))

            # ---- phase X: streamed x chunks, stage-3 chase, stacked emit ------
            xgs = []
            for b in range(BPC):
                for g in range(NG):
                    xg = pXg.tile([128, GCH, T], BF16, tag="xg", name=f"xg{b}_{g}")
                    nc.sync.dma_start(
                        xg, xbT[b, g * GCH * 128:(g + 1) * GCH * 128, :]
                        .rearrange("(c p) t -> p c t", p=128))
                    xgs.append(xg)

            with tc.tile_pool(name="pXw", bufs=6) as pXw, \
                 tc.tile_pool(name="pY", bufs=2) as pY, \
                 tc.tile_pool(name="obuf4", bufs=4) as obuf4, \
                 tc.tile_pool(name="yp", bufs=2, space="PSUM") as yp, \
                 tc.tile_pool(name="op", bufs=4, space="PSUM") as op:
                for b in range(BPC):
                    yk0 = pY.tile([128, T], BF16, tag="yk0", name=f"yk0_{b}")
                    yk1 = pY.tile([128, T], BF16, tag="yk1", name=f"yk1_{b}")
                    ymix = pY.tile([R + 1, T], BF16, tag="ymix", name=f"ymix_{b}")
                    nc.vector.memset(ymix[R:R + 1, :], 1.0)

                    # stage 3: y^T[r, t] accumulated over 32 nc-chunks in 4 groups
                    py = [yp.tile([R, HALF], F32, tag="y", name=f"py{b}_{i}")
                          for i in range(2)]
                    for g in range(NG):
                        for hf in range(2):
                            for c in range(GCH):
                                ch = g * GCH + c
                                nc.tensor.matmul(
                                    py[hf],
                                    projf[:, b * CH + (ch % 2) * N + ch // 2, :],
                                    xgs[b * NG + g][:, c,
                                                    hf * HALF:(hf + 1) * HALF],
                                    start=(ch == 0), stop=(ch == CH - 1))
                    for hf in range(2):
                        nc.vector.tensor_copy(
                            ysb[b][:, PAD + hf * HALF:PAD + (hf + 1) * HALF],
                            py[hf])
                        # build the shifted stacks as each y half lands: part 1
                        # [0, YSPL) only needs half 0, part 2 needs half 1 too
                        lo = 0 if hf == 0 else YSPL
                        hi = YSPL if hf == 0 else T
                        for q, (_kb, off) in enumerate(JOFF[0:4]):
                            nc.gpsimd.dma_start(
                                yk0[q * R:(q + 1) * R, lo:hi],
                                ysb[b][:, PAD + lo + off:PAD + hi + off])
                        for q, (_kb, off) in enumerate(JOFF[4:8]):
                            nc.gpsimd.dma_start(
                                yk1[q * R:(q + 1) * R, lo:hi],
                                ysb[b][:, PAD + lo + off:PAD + hi + off])
                        nc.gpsimd.dma_start(
                            ymix[0:R, lo:hi],
                            ysb[b][:, PAD + lo + 2:PAD + hi + 2])

                    # emit: 3 stacked matmuls (incl. bo) + LN + relu per tile
                    for ts in range(T // 128):
                        po = op.tile([128, C], F32, tag="o")
                        nc.tensor.matmul(po, yk0[:, ts * 128:(ts + 1) * 128],
                                         mst0, start=True, stop=False)
                        nc.tensor.matmul(po, yk1[:, ts * 128:(ts + 1) * 128],
                                         mst1, start=False, stop=False)
                        nc.tensor.matmul(po, ymix[:, ts * 128:(ts + 1) * 128],
                                         mcat, start=False, stop=True)
                        st = pXw.tile([128, 6], F32, tag="st")
                        nc.vector.bn_stats(out=st, in_=po)
                        mv = pXw.tile([128, 2], F32, tag="mv")
                        nc.vector.bn_aggr(out=mv, in_=st)
                        rs = pXw.tile([128, 1], F32, tag="rs")
                        nc.scalar.activation(out=rs, in_=mv[:, 1:2], func=SQRT,
                                             bias=epsb, scale=1.0)
                        nc.vector.reciprocal(rs, rs)
                        zn = pXw.tile([128, C], BF16, tag="zn")
                        nc.vector.tensor_scalar(zn, po, mv[:, 0:1], rs,
                                                mybir.AluOpType.subtract,
                                                mybir.AluOpType.mult)
                        zg = pXw.tile([128, C], BF16, tag="zg")
                        nc.vector.tensor_mul(zg, zn, gb16)
                        nc.vector.tensor_add(zg, zg, bb16)
                        if ts % 4 == 0:
                            ob = obuf4.tile([128, 4, C], BF16, tag="ob4",
                                            name=f"ob{b}_{ts}")
                        nc.vector.tensor_scalar_max(ob[:, ts % 4, :], zg, 0.0)
                        if ts % 4 == 3:
                            # scalar-engine DMA queue: stores bypass the big
                            # x-stream FIFO on the sync queue
                            nc.scalar.dma_start(
                                out[b, ts // 4, :, :]
                                .rearrange("p (q c) -> p q c", q=4), ob)

    nc.compile()
    return nc


_NC = None


def _get_nc():
    global _NC
    if _NC is None:
        _NC = _build()
    return _NC


def _shard(inputs):
    """Split full inputs into per-core input maps (slicing/transpose/cast only)."""
    x = np.asarray(inputs["context_emb"], dtype=np.float32)
    B = x.shape[0]
    assert B == NCORES * BPC
    # pre-transpose per sample to [NCF, T] and cast bf16 (layout prep on host)
    xT = np.ascontiguousarray(
        np.swapaxes(x.reshape(B, T, NCF), 1, 2)).astype(BF)
    ph = np.asarray(inputs["phrase_slot"], dtype=np.float32)
    kjoin = np.concatenate(
        [np.moveaxis(inputs[f"k{k}"], 2, 0) for k in (1, 3, 5)],
        axis=0).astype(BF)  # [9, 32, 256]
    shared = {
        "eos": np.asarray(inputs["eos_slot"], dtype=np.float32).reshape(C).astype(BF),
        "w1": np.asarray(inputs["W1"], dtype=np.float32).astype(BF),
        "b1": np.ascontiguousarray(inputs["b1"], dtype=np.float32),
        "w2": np.asarray(inputs["W2"], dtype=np.float32).astype(BF),
        "b2": np.ascontiguousarray(inputs["b2"], dtype=np.float32),
        "kjoin": np.ascontiguousarray(kjoin),
        "wo": np.asarray(inputs["Wo"], dtype=np.float32).astype(BF),
        "lnp": np.ascontiguousarray(np.stack([
            np.asarray(inputs["gamma"], dtype=np.float32),
            np.asarray(inputs["beta"], dtype=np.float32),
            np.asarray(inputs["bo"], dtype=np.float32)])),
    }
    in_maps = []
    for i in range(NCORES):
        m = dict(shared)
        m["xbT"] = np.ascontiguousarray(xT[i * BPC:(i + 1) * BPC])
        m["phrase"] = np.ascontiguousarray(
            ph[i * BPC:(i + 1) * BPC].reshape(BPC * N, C).astype(BF))
        in_maps.append(m)
    return in_maps


def _run(inputs, **kwargs):
    nc = _get_nc()
    res = run_bass_kernel_spmd(nc, _shard(inputs), core_ids=list(range(NCORES)),
                               **kwargs)
    outs = [r["out"] for r in res.results]
    full = np.concatenate(outs, axis=0).reshape(NCORES * BPC, 2, 128, 4, C)
    # [b, s, p, q, c] -> t = (s*4 + q)*128 + p
    full = np.ascontiguousarray(full.transpose(0, 1, 3, 2, 4)).reshape(
        NCORES * BPC, T, C)
    return full.astype(np.float32), res


def kernel(**inputs) -> np.ndarray:
    out, _ = _run(inputs)
    return out
